# revision 10
# baseline (speedup 1.0000x reference)
"""Multi-head attention (B=4, S=2048, D=1024, H=16) + output projection on 8 trn2 cores.

Sharding: no collectives. Core c handles batch c//2, query rows (c%2)*1024..+1024,
all 16 heads. Each core needs full K/V for its batch; W_out/b_out replicated.
The per-core output block [1024, 1024] is the final projected output for those
query rows, so the host just concatenates.

Per-core pipeline (all matmuls bf16, fp32 PSUM accumulation):
  - q/k cast fp32->bf16 by SWDGE straight into SBUF in column chunks
    (128 | 896, k's wide chunk split into two row halves so head pair 1 can
    start early), then HWDGE SBUF->SBUF xbar transposes put head_dim on
    partitions. No DRAM staging round trip for q/k; W takes the DRAM route
    late in the kernel when the queues are idle.
  - v loads whole bf16 rows into SBUF in 2-jc chunks (2KB descriptors), then
    cheap DVE copies scatter each chunk into the per-head v_aug layout
    [j, h, jc, 66] whose col 64 is ones (softmax sums for free) and col 65
    pads for DVE-mode alignment.
  - attention per head pair hp: per j-chunk, the 4 score matmuls issue
    h2-interleaved with tile_position row packing so the two 64-contraction
    matmuls stream concurrently in disjoint PE row groups; exp runs EITHER on
    ScalarE (activation Exp) OR on DVE (Schraudolph bit-trick: one fused
    tensor_scalar mult+add with int16 output whose bits ARE the bf16 exp
    value) splitting ~290us of exp work across two engines; AV matmuls
    (v_aug weights) are deferred one j-chunk so they never head-of-line-block
    the PE queue on an exp.
  - normalization: one DVE copy [65,1024] frees the AV psum, fast reciprocal
    of the sums row, gpsimd partition_broadcast, and a one-hp-deferred DVE
    multiply produce attT bf16.
  - projection: final[i, e] = attT.T @ wT over 8 d-chunks, bias added on DVE.
"""

import numpy as np

import concourse.bass as bass
import concourse.tile as tile
from concourse import bacc, mybir
from concourse.bass_utils import run_bass_kernel_spmd

B = 4
S = 2048
DM = 1024
H = 16
DK = 64
SCALE = DK**-0.5
I = 1024  # local query rows per core
NJC = S // 128  # 16 j-chunks
NHP = H // 2  # 8 head pairs == 8 d-chunks of the model dim

F32 = mybir.dt.float32
BF16 = mybir.dt.bfloat16
I16 = mybir.dt.int16

# Schraudolph exp in bf16: bits = trunc(A*x + B); A folds the softmax scale,
# B = 128*127 - c with c tuned for truncating float->int16 conversion.
A_EXP = float(128.0 / np.log(2.0) * SCALE)
B_EXP = float(128.0 * 127.0 - 5.5)

USE_DVE_EXP = True  # hybrid exp bisect toggle
USE_INTERLEAVE = True  # h2-interleaved scores + deferred AV bisect toggle
DEBUG_DUMP = True


def build(nc: bass.Bass):
    q = nc.dram_tensor("q", [I, DM], F32, kind="ExternalInput").ap()
    k = nc.dram_tensor("k", [S, DM], F32, kind="ExternalInput").ap()
    v = nc.dram_tensor("v", [S, DM], F32, kind="ExternalInput").ap()
    w = nc.dram_tensor("w", [DM, DM], F32, kind="ExternalInput").ap()
    b = nc.dram_tensor("b", [DM], F32, kind="ExternalInput").ap()
    out = nc.dram_tensor("out", [I, DM], F32, kind="ExternalOutput").ap()
    if DEBUG_DUMP:
        dbg_qT = nc.dram_tensor("dbg_qT", [128, 8, I], BF16, kind="ExternalOutput").ap()
        dbg_kT = nc.dram_tensor("dbg_kT", [128, 8, S], BF16, kind="ExternalOutput").ap()
        dbg_vA = nc.dram_tensor("dbg_vA", [128, H * NJC * 66], BF16, kind="ExternalOutput").ap()
        dbg_att = nc.dram_tensor("dbg_att", [NHP, 128, I], BF16, kind="ExternalOutput").ap()
        dbg_wT = nc.dram_tensor("dbg_wT", [128, NHP, DM], BF16, kind="ExternalOutput").ap()

    q_bf0 = nc.dram_tensor("q_bf0", [I, 128], BF16).ap()
    q_bf1 = nc.dram_tensor("q_bf1", [I, 896], BF16).ap()
    k_bf0 = nc.dram_tensor("k_bf0", [S, 128], BF16).ap()
    k_bf1 = nc.dram_tensor("k_bf1", [S, 896], BF16).ap()
    w_bf = nc.dram_tensor("w_bf", [DM, DM], BF16).ap()

    with tile.TileContext(nc) as tc:
        with (
            tc.tile_pool(name="persist", bufs=1) as pers,
            tc.tile_pool(name="vstg", bufs=3) as vstg,
            tc.tile_pool(name="expp", bufs=6) as expp,
            tc.tile_pool(name="avsbp", bufs=4) as avsbp,
            tc.tile_pool(name="nrmp", bufs=4) as nrmp,
            tc.tile_pool(name="finp", bufs=2) as finp,
        ):
            # ---- PE warmup: dummy matmuls so HAM un-throttles during the
            # DMA prelude (zeroed input; results never read) ----
            warm_sb = pers.tile([128, 512], BF16, name="warm_sb", tag="warm_sb")
            nc.vector.memset(warm_sb[:, :], 0.0)

            # ---- v: bf16 rows into SBUF in 2-jc chunks, DVE-scattered into
            # the aug layout [j%128, h, jc, 66] (col 64 = ones) ----
            vA_all = pers.tile([128, H * NJC * 66], BF16, name="vA_all", tag="vA_all")
            vA4 = vA_all[:, :].rearrange("p (h jc e) -> p h jc e", h=H, e=66)
            nc.vector.memset(vA4[:, :, :, DK], 1.0)

            v_chunks = {}

            def load_v_chunk(c):  # jc pair (2c, 2c+1)
                t = vstg.tile([128, 2, DM], BF16, name=f"v_c{c}", tag="v_c")
                src = v[c * 256 : (c + 1) * 256, :].rearrange("(jc p) d -> p jc d", p=128)
                nc.gpsimd.dma_start(out=t[:, :, :], in_=src)
                v_chunks[c] = t

            def scatter_v_jc(jc):
                t = v_chunks[jc // 2]
                nc.vector.tensor_copy(
                    vA4[:, :, jc, 0:DK],
                    t[:, jc % 2, :].rearrange("p (h d) -> p h d", d=DK),
                )

            def vA(h):
                return vA_all[:, h * NJC * 66 : (h + 1) * NJC * 66]

            # ---- q/k fp32->bf16 DRAM staging (SWDGE cast, full-row reads) ----
            bias_sb = pers.tile([1, DM], BF16, name="bias_sb", tag="bias_sb")
            nc.gpsimd.dma_start(out=bias_sb[:, :], in_=b[None, :])
            nc.gpsimd.dma_start(out=q_bf0[:, :], in_=q[:, 0:128])
            nc.gpsimd.dma_start(out=k_bf0[:, :], in_=k[:, 0:128])
            load_v_chunk(0)
            load_v_chunk(1)
            load_v_chunk(2)

            def gp_prefetch(hp):
                # emitted inside the hp loop so the GpSimd FIFO drains between
                # bulk DMA descriptor generation and the normalize broadcasts
                if hp == 0:
                    nc.gpsimd.dma_start(out=q_bf1[:, :], in_=q[:, 128:1024])
                    nc.gpsimd.dma_start(out=k_bf1[0:1024, :], in_=k[0:1024, 128:1024])
                    for c in range(3, 8):
                        load_v_chunk(c)
                    nc.sync.dma_start(
                        out=qT_b[:, :, :], in_=q_bf1[:, :], transpose=True
                    )
                    nc.sync.dma_start(
                        out=kT_b1[:, :, :], in_=k_bf1[0:1024, :], transpose=True
                    )
                elif hp == 1:
                    nc.gpsimd.dma_start(
                        out=k_bf1[1024:2048, :], in_=k[1024:2048, 128:1024]
                    )
                    nc.sync.dma_start(
                        out=kT_b2[:, :, :], in_=k_bf1[1024:2048, :], transpose=True
                    )
                elif hp == 4:
                    nc.gpsimd.dma_start(out=w_bf[:, :], in_=w[:, :])
                elif hp == 5:
                    nc.sync.dma_start(
                        out=wT_all[:, :, :], in_=w_bf[:, :], transpose=True
                    )

            # ---- transposed operands (HWDGE xbar: out[p, e, r] = in[r, e*128+p]) ----
            qT_a = pers.tile([128, 1, I], BF16, name="qT_a", tag="qT_a")
            qT_b = pers.tile([128, 7, I], BF16, name="qT_b", tag="qT_b")
            kT_a = pers.tile([128, 1, S], BF16, name="kT_a", tag="kT_a")
            kT_b1 = pers.tile([128, 7, 1024], BF16, name="kT_b1", tag="kT_b1")
            kT_b2 = pers.tile([128, 7, 1024], BF16, name="kT_b2", tag="kT_b2")
            wT_all = pers.tile([128, NHP, DM], BF16, name="wT_all", tag="wT_all")
            bias_bc = pers.tile([128, DM], BF16, name="bias_bc", tag="bias_bc")
            nc.sync.dma_start(out=qT_a[:, :, :], in_=q_bf0[:, :], transpose=True)
            nc.sync.dma_start(out=kT_a[:, :, :], in_=k_bf0[:, :], transpose=True)

            def qT(hp, h2, ih):
                # [64 d, 512 i] rhs slice
                if hp == 0:
                    return qT_a[h2 * DK : (h2 + 1) * DK, 0, ih * 512 : (ih + 1) * 512]
                return qT_b[h2 * DK : (h2 + 1) * DK, hp - 1, ih * 512 : (ih + 1) * 512]

            def kT(hp, h2, jc):
                # [64 d, 128 j] lhsT slice
                if hp == 0:
                    return kT_a[h2 * DK : (h2 + 1) * DK, 0, jc * 128 : (jc + 1) * 128]
                if jc < 8:
                    return kT_b1[
                        h2 * DK : (h2 + 1) * DK, hp - 1, jc * 128 : (jc + 1) * 128
                    ]
                return kT_b2[
                    h2 * DK : (h2 + 1) * DK, hp - 1, (jc - 8) * 128 : (jc - 7) * 128
                ]

            def wT(dc):
                return wT_all[:, dc, :]

            attT = []
            for hp in range(NHP):
                at = pers.tile([128, I], BF16, name=f"attT{hp}", tag=f"attT{hp}")
                attT.append(at)

            # warmup matmuls (no data deps beyond the memset)
            with tc.tile_pool(name="warmp", bufs=1, space="PSUM") as warmp:
                wps = warmp.tile([128, 512], F32, name="wps", tag="wps")
                for _ in range(14):
                    nc.tensor.matmul(
                        wps[:, :],
                        warm_sb[:, 0:128],
                        warm_sb[:, :],
                        start=True,
                        stop=True,
                        skip_group_check=True,
                    )

            # ---- attention: hybrid-exp pipeline ----
            with (
                tc.tile_pool(name="smmp", bufs=2, space="PSUM") as smmp,
                tc.tile_pool(name="avp", bufs=2, space="PSUM") as avp,
            ):
                pending_mults = []
                for hp in range(NHP):
                    gp_prefetch(hp)
                    if hp == 0:
                        for jc in range(NJC):
                            scatter_v_jc(jc)
                    cur = []
                    av = [
                        avp.tile([65, I], F32, name=f"av{hp}_{h2}", tag="av")
                        for h2 in range(2)
                    ]
                    pending_av = None  # (jc, [rhs_h2_0, rhs_h2_1])
                    if not USE_INTERLEAVE:
                        # baseline-style: per (jc, h2): scores, exp, AV inline
                        for jc in range(NJC):
                            for h2 in range(2):
                                smm1 = smmp.tile(
                                    [128, I], F32, name=f"smm{hp}_{jc}_{h2}", tag="smm"
                                )
                                for ih in range(2):
                                    nc.tensor.matmul(
                                        smm1[:, ih * 512 : (ih + 1) * 512],
                                        kT(hp, h2, jc),
                                        qT(hp, h2, ih),
                                        start=True,
                                        stop=True,
                                        tile_position=(h2 * DK, 0),
                                    )
                                expt = expp.tile(
                                    [128, I], BF16, name=f"ex{hp}_{jc}_{h2}", tag="expt"
                                )
                                nc.scalar.activation(
                                    expt[:, :],
                                    smm1[:, :],
                                    mybir.ActivationFunctionType.Exp,
                                    scale=SCALE,
                                )
                                h = 2 * hp + h2
                                for ih in range(2):
                                    nc.tensor.matmul(
                                        av[h2][:, ih * 512 : (ih + 1) * 512],
                                        vA(h)[:, jc * 66 : jc * 66 + 65],
                                        expt[:, ih * 512 : (ih + 1) * 512],
                                        start=(jc == 0),
                                        stop=(jc == NJC - 1),
                                        skip_group_check=True,
                                    )
                    for jc in range(NJC if USE_INTERLEAVE else 0):
                        # scores: interleave h2 so the two 64-row matmuls
                        # stream concurrently in disjoint PE row groups
                        smm = [
                            smmp.tile([128, I], F32, name=f"smm{hp}_{jc}_{h2}", tag="smm")
                            for h2 in range(2)
                        ]
                        for ih in range(2):
                            for h2 in range(2):
                                nc.tensor.matmul(
                                    smm[h2][:, ih * 512 : (ih + 1) * 512],
                                    kT(hp, h2, jc),
                                    qT(hp, h2, ih),
                                    start=True,
                                    stop=True,
                                    tile_position=(h2 * DK, 0),
                                )
                        # exp on ScalarE or DVE (Schraudolph bit-trick); hp 0
                        # stays all-ScalarE so the v scatters (which wait on v
                        # chunk DMAs) never head-of-line-block DVE exps
                        rhs = []
                        for h2 in range(2):
                            if USE_DVE_EXP and hp > 0 and h2 == 1 and jc % 4 != 3:
                                eti = expp.tile(
                                    [128, I], I16, name=f"exi{hp}_{jc}_{h2}", tag="expt"
                                )
                                nc.vector.tensor_scalar(
                                    eti[:, :],
                                    smm[h2][:, :],
                                    A_EXP,
                                    B_EXP,
                                    mybir.AluOpType.mult,
                                    mybir.AluOpType.add,
                                )
                                rhs.append(eti[:, :].bitcast(BF16))
                            else:
                                expt = expp.tile(
                                    [128, I], BF16, name=f"ex{hp}_{jc}_{h2}", tag="expt"
                                )
                                nc.scalar.activation(
                                    expt[:, :],
                                    smm[h2][:, :],
                                    mybir.ActivationFunctionType.Exp,
                                    scale=SCALE,
                                )
                                rhs.append(expt[:, :])
                        # AV matmuls deferred one j-chunk: their exp is long
                        # done, so they never stall the PE queue
                        if pending_av is not None:
                            pjc, prhs = pending_av
                            for h2 in range(2):
                                h = 2 * hp + h2
                                for ih in range(2):
                                    nc.tensor.matmul(
                                        av[h2][:, ih * 512 : (ih + 1) * 512],
                                        vA(h)[:, pjc * 66 : pjc * 66 + 65],
                                        prhs[h2][:, ih * 512 : (ih + 1) * 512],
                                        start=(pjc == 0),
                                        stop=False,
                                        skip_group_check=True,
                                    )
                        pending_av = (jc, rhs)
                    # flush last j-chunk's AV
                    if pending_av is None:
                        pending_av = (None, None)
                    pjc, prhs = pending_av
                    if pjc is not None:
                        for h2 in range(2):
                            h = 2 * hp + h2
                            for ih in range(2):
                                nc.tensor.matmul(
                                    av[h2][:, ih * 512 : (ih + 1) * 512],
                                    vA(h)[:, pjc * 66 : pjc * 66 + 65],
                                    prhs[h2][:, ih * 512 : (ih + 1) * 512],
                                    start=False,
                                    stop=True,
                                    skip_group_check=True,
                                )
                    # one copy frees the av psum; reciprocal + broadcast
                    # follow off the critical path
                    for h2 in range(2):
                        asb = avsbp.tile([DK, I], F32, name=f"avsb{hp}_{h2}", tag="avsb")
                        nc.vector.tensor_copy(asb[:, :], av[h2][0:DK, :])
                        sums = nrmp.tile([1, I], F32, name=f"sm{hp}_{h2}", tag="sums", bufs=2)
                        nc.vector.tensor_copy(sums[:, :], av[h2][DK : DK + 1, :])
                        r = nrmp.tile([1, I], F32, name=f"rc{hp}_{h2}", tag="rc", bufs=2)
                        nc.vector.reciprocal_approx_fast(r[:, :], sums[:, :])
                        rb = nrmp.tile([DK, I], F32, name=f"rb{hp}_{h2}", tag="rb")
                        nc.gpsimd.partition_broadcast(rb[:, :], r[0:1, :])
                        cur.append((asb, rb))
                    # normalize multiplies deferred one head pair: their
                    # broadcast is then long done, so they never head-of-line
                    # block the DVE FIFO
                    for h2, (asb, rb) in enumerate(pending_mults):
                        nc.vector.tensor_mul(
                            attT[hp - 1][h2 * DK : (h2 + 1) * DK, :],
                            asb[:, :],
                            rb[:, :],
                        )
                    pending_mults = cur
                for h2, (asb, rb) in enumerate(pending_mults):
                    nc.vector.tensor_mul(
                        attT[NHP - 1][h2 * DK : (h2 + 1) * DK, :],
                        asb[:, :],
                        rb[:, :],
                    )

                # keep PE warm across the normalize -> projection handoff
                wps2 = smmp.tile([128, I], F32, name="wps2", tag="smm")
                for _ in range(10):
                    nc.tensor.matmul(
                        wps2[:, 0:512],
                        warm_sb[:, 0:128],
                        warm_sb[:, :],
                        start=True,
                        stop=True,
                        skip_group_check=True,
                    )

            # bias broadcast emitted late so it doesn't block the hp-loop
            # normalize broadcasts in the GpSimd FIFO
            nc.gpsimd.partition_broadcast(bias_bc[:, :], bias_sb[0:1, :])

            if DEBUG_DUMP:
                nc.sync.dma_start(out=dbg_qT[:, 0:1, :], in_=qT_a[:, :, :])
                nc.sync.dma_start(out=dbg_qT[:, 1:8, :], in_=qT_b[:, :, :])
                nc.sync.dma_start(out=dbg_kT[:, 0:1, :], in_=kT_a[:, :, :])
                nc.sync.dma_start(
                    out=dbg_kT[:, 1:8, 0:1024], in_=kT_b1[:, :, :]
                )
                nc.sync.dma_start(
                    out=dbg_kT[:, 1:8, 1024:2048], in_=kT_b2[:, :, :]
                )
                nc.sync.dma_start(out=dbg_vA[:, :], in_=vA_all[:, :])
                for hp in range(NHP):
                    nc.sync.dma_start(out=dbg_att[hp, :, :], in_=attT[hp][:, :])
                nc.sync.dma_start(out=dbg_wT[:, :, :], in_=wT_all[:, :, :])

            # ---- output projection ----
            with tc.tile_pool(name="projp", bufs=4, space="PSUM") as projp:
                for ic in range(I // 128):
                    # interleave the two ec psum banks so consecutive
                    # accumulating matmuls alternate banks and pipeline
                    pp = [
                        projp.tile([128, 512], F32, name=f"pp{ic}_{ec}", tag="pp")
                        for ec in range(2)
                    ]
                    for dc in range(NHP):
                        for ec in range(2):
                            nc.tensor.matmul(
                                pp[ec][:, :],
                                attT[dc][:, ic * 128 : (ic + 1) * 128],
                                wT(dc)[:, ec * 512 : (ec + 1) * 512],
                                start=(dc == 0),
                                stop=(dc == NHP - 1),
                                skip_group_check=True,
                            )
                    for ec in range(2):
                        fin = finp.tile([128, 512], F32, name=f"fin{ic}_{ec}", tag="fin")
                        nc.vector.tensor_add(
                            fin[:, :], pp[ec][:, :], bias_bc[:, ec * 512 : (ec + 1) * 512]
                        )
                        nc.sync.dma_start(
                            out=out[
                                ic * 128 : (ic + 1) * 128, ec * 512 : (ec + 1) * 512
                            ],
                            in_=fin[:, :],
                        )
    return nc


_NC_CACHE = {}


def _get_nc():
    if "nc" not in _NC_CACHE:
        nc = bacc.Bacc("TRN2", target_bir_lowering=False, debug=False)
        build(nc)
        nc.compile()
        _NC_CACHE["nc"] = nc
    return _NC_CACHE["nc"]


def kernel(q, k, v, W_out, b_out, _trace=False, _trace_kwargs=None):
    q = np.asarray(q, dtype=np.float32)
    k = np.asarray(k, dtype=np.float32)
    v = np.asarray(v, dtype=np.float32)
    W_out = np.ascontiguousarray(np.asarray(W_out, dtype=np.float32))
    b_out = np.ascontiguousarray(np.asarray(b_out, dtype=np.float32))

    nc = _get_nc()
    in_maps = []
    for c in range(8):
        bi, half = c // 2, c % 2
        in_maps.append(
            {
                "q": np.ascontiguousarray(q[bi, half * I : (half + 1) * I, :]),
                "k": np.ascontiguousarray(k[bi]),
                "v": np.ascontiguousarray(v[bi]),
                "w": W_out,
                "b": b_out,
            }
        )
    res = run_bass_kernel_spmd(
        nc,
        in_maps,
        core_ids=list(range(8)),
        trace=_trace,
        **(_trace_kwargs or {}),
    )
    out = np.empty((B, S, DM), np.float32)
    for c in range(8):
        bi, half = c // 2, c % 2
        out[bi, half * I : (half + 1) * I, :] = res.results[c]["out"]
    if _trace:
        return out, res
    return out
